# revision 1
# baseline (speedup 1.0000x reference)
"""Multi-head attention on 8 NeuronCores (Trainium2, Bass/Tile).

Problem: B=2, S=2048, E=1024, H=16, D=64 MHA with int mask, fp32.

Sharding (per the tensor-parallel hint): core c = 4*b + g handles batch b,
head group g (4 heads = a 256-wide slice of E).  Q/K/V projections, scores,
softmax and attention are head-parallel; Wo is row-sharded so each core
emits a partial [S, E] output projection; the host sums the 4 partials per
batch (the all-reduce) and adds bo.

Device pipeline per core (S=2048, local j = h*64+d in [0,256)):
  qhT, khT : [j, S] fp32r (pair-major [128, pair, S]); produced by PE from
             fp16 x-transposed streams and fp16 weights (fp32 accumulate).
  vh       : [S, j] as [128, s_tile, head, 65] fp16 with a ones column ->
             P @ [vh|1] yields the softmax denominator for free.
  scores_T : [ks, q] fp32r matmuls into PSUM (K=64; head pairs land on
             different PE row groups so they overlap on HW); ACT exp
             (scale=1/8) -> fp16; DVE mask multiply (fp16 2x mode);
             PE accumulates ctx_T over ks (fp32 PSUM).
  ctx_T    : normalized via reciprocal + partition_broadcast, stored
             [j, S] fp32r; partial out = ctx_T.T @ WoT on PE, fp16 DMA out.
"""

import os
import sys

sys.path.insert(0, "/opt/trn_rl_repo")

import numpy as np

import concourse.mybir as mybir
import concourse.tile as tile
from concourse import bacc
from concourse import bass_utils

B, S, E, H = 2, 2048, 1024, 16
D = E // H              # 64
G = 4                   # head groups (cores per batch)
HL = H // G             # 4 local heads per core
J = HL * D              # 256 local j width
P = 128
KT = E // P             # 8 k-tiles for projections
ST = S // P             # 16 s-tiles / ks-tiles
NQ = 1024               # q-chunk width for attention
QC = S // NQ            # 2 q chunks
MC = 4                  # mask ks-tiles per DMA chunk

F32 = mybir.dt.float32
F32R = mybir.dt.float32r
F16 = mybir.dt.float16

# Exposed for test.py / bench.py.
LAST_RESULTS = None
LAST_NC = None


def _round_f32r(x: np.ndarray) -> np.ndarray:
    """Round fp32 to fp32r (tf32-like, keep 10 mantissa bits), RNE."""
    u = np.ascontiguousarray(x, dtype=np.float32).view(np.uint32)
    u = (u + 0x00000FFF + ((u >> 13) & 1)) & 0xFFFFE000
    return u.astype(np.uint32).view(np.float32)


def _f16(x: np.ndarray) -> np.ndarray:
    return np.ascontiguousarray(x, dtype=np.float32).astype(np.float16)


_bf16 = _f16  # kept for external callers (test.py / bench scripts)


def _build_program(use_bias_qk: bool, use_bias_v: bool):
    nc = bacc.Bacc("TRN2", target_bir_lowering=False, debug=False, num_devices=8)

    xqT = nc.dram_tensor("xqT", [E, S], F16, kind="ExternalInput")
    xkT = nc.dram_tensor("xkT", [E, S], F16, kind="ExternalInput")
    xvT = nc.dram_tensor("xvT", [E, S], F16, kind="ExternalInput")
    maskT = nc.dram_tensor("maskT", [S, S], F16, kind="ExternalInput")
    wqT = nc.dram_tensor("wqT", [E, J], F16, kind="ExternalInput")
    wkT = nc.dram_tensor("wkT", [E, J], F16, kind="ExternalInput")
    wvT = nc.dram_tensor("wvT", [E, J], F16, kind="ExternalInput")
    woT = nc.dram_tensor("woT", [J, E], F32R, kind="ExternalInput")
    bq = nc.dram_tensor("bq", [J], F32, kind="ExternalInput")
    bk = nc.dram_tensor("bk", [J], F32, kind="ExternalInput")
    bv = nc.dram_tensor("bv", [J], F32, kind="ExternalInput")
    out = nc.dram_tensor("out", [S, E], F16, kind="ExternalOutput")

    Copy = mybir.ActivationFunctionType.Copy
    Exp = mybir.ActivationFunctionType.Exp

    with tile.TileContext(nc) as tc:
        with (
            tc.tile_pool(name="consts", bufs=1) as consts,
            tc.tile_pool(name="persist", bufs=1) as persist,
            tc.tile_pool(name="xs", bufs=5) as xs,
            tc.tile_pool(name="xv", bufs=1) as xvpool,
            tc.tile_pool(name="maskp", bufs=4) as maskp,
            tc.tile_pool(name="pwork", bufs=6) as pwork,
            tc.tile_pool(name="osb", bufs=4) as osb,
            tc.tile_pool(name="small", bufs=2) as small,
        ):
            # ---- weights / constants ----
            wq_sb = consts.tile([P, KT, J], F16, tag="wq")
            wk_sb = consts.tile([P, KT, J], F16, tag="wk")
            wv_sb = consts.tile([P, KT, J], F16, tag="wv")
            wo_sb = consts.tile([P, J // P, E], F32R, tag="wo")

            if use_bias_qk:
                bq_sb = consts.tile([P, J // P], F32, tag="bq")
                bk_sb = consts.tile([P, J // P], F32, tag="bk")
                nc.sync.dma_start(bq_sb[:], bq.rearrange("(pr p) -> p pr", p=P))
                nc.sync.dma_start(bk_sb[:], bk.rearrange("(pr p) -> p pr", p=P))
            if use_bias_v:
                bv_row = consts.tile([1, J], F32, tag="bvr")
                nc.sync.dma_start(bv_row[:], bv.rearrange("j -> 1 j"))
                bv_bc = consts.tile([P, J], F32, tag="bvb")
                nc.gpsimd.partition_broadcast(bv_bc[:], bv_row[:])

            # ---- persistent activations ----
            qhT = persist.tile([P, 2, S], F32R, tag="qhT")
            khT = persist.tile([P, 2, S], F32R, tag="khT")
            vh = persist.tile([P, ST, HL, 65], F16, tag="vh")
            ctxT = persist.tile([P, 2, S], F32R, tag="ctxT")

            nc.gpsimd.memset(vh[:, :, :, 64:65], 1.0)

            # ---- phase A: projections ----
            projacc_cm = tc.tile_pool(name="projacc", bufs=8, space="PSUM")
            projacc = projacc_cm.__enter__()

            nc.sync.dma_start(wq_sb[:], wqT.rearrange("(kt p) j -> p kt j", p=P))
            nc.sync.dma_start(wk_sb[:], wkT.rearrange("(kt p) j -> p kt j", p=P))

            # q and k -> transposed layout [j, s], pair-major
            for w_sb, x_dram, outT, b_sb in (
                (wq_sb, xqT, qhT, "bq"),
                (wk_sb, xkT, khT, "bk"),
            ):
                accs = [projacc.tile([P, 512], F32, tag="pacc", name=f"pacc{i}")
                        for i in range(8)]
                for kt in range(KT):
                    xt = xs.tile([P, S], F16, tag="xt")
                    nc.sync.dma_start(xt[:], x_dram[kt * P:(kt + 1) * P, :])
                    for pair in range(2):
                        for n4 in range(4):
                            nc.tensor.matmul(
                                accs[pair * 4 + n4][:],
                                w_sb[:, kt, pair * P:(pair + 1) * P],
                                xt[:, n4 * 512:(n4 + 1) * 512],
                                start=(kt == 0), stop=(kt == KT - 1),
                            )
                for pair in range(2):
                    for n4 in range(4):
                        dst = outT[:, pair, n4 * 512:(n4 + 1) * 512]
                        src = accs[pair * 4 + n4][:]
                        if use_bias_qk:
                            bias = (bq_sb if b_sb == "bq" else bk_sb)[:, pair:pair + 1]
                            nc.scalar.activation(dst, src, Copy, bias=bias)
                        elif n4 % 2 == 0:
                            nc.vector.tensor_copy(dst, src)
                        else:
                            nc.scalar.activation(dst, src, Copy)
            # v -> natural layout [s, j]; x_v tiles stay resident, two psum
            # half-passes of 8 s-tiles (one bank per accumulator).
            nc.sync.dma_start(wv_sb[:], wvT.rearrange("(kt p) j -> p kt j", p=P))
            xvt = [xvpool.tile([P, S], F16, tag=f"xv{i}", name=f"xv{i}")
                   for i in range(KT)]
            for kt in range(KT):
                nc.sync.dma_start(xvt[kt][:], xvT[kt * P:(kt + 1) * P, :])
            for sh in range(2):
                vaccs = [projacc.tile([P, J], F32, tag="pacc", name=f"vacc{sh}_{i}")
                         for i in range(8)]
                for kt in range(KT):
                    for si in range(8):
                        st = sh * 8 + si
                        nc.tensor.matmul(
                            vaccs[si][:],
                            xvt[kt][:, st * P:(st + 1) * P],
                            wv_sb[:, kt, :],
                            start=(kt == 0), stop=(kt == KT - 1),
                        )
                for si in range(8):
                    st = sh * 8 + si
                    src3 = vaccs[si][:].rearrange("p (h d) -> p h d", h=HL)
                    dst = vh[:, st, :, 0:64]
                    if use_bias_v:
                        nc.vector.tensor_add(
                            dst, src3, bv_bc[:].rearrange("p (h d) -> p h d", h=HL)
                        )
                    elif si % 2 == 0:
                        nc.vector.tensor_copy(dst, src3)
                    else:
                        nc.scalar.activation(dst, src3, Copy)

            projacc_cm.__exit__(None, None, None)

            # ---- phase B: attention ----
            stps_cm = tc.tile_pool(name="stps", bufs=2, space="PSUM")
            stps = stps_cm.__enter__()
            ctxps_cm = tc.tile_pool(name="ctxps", bufs=4, space="PSUM")
            ctxps = ctxps_cm.__enter__()
            for qc in range(QC):
                mtiles = {}
                for hp in range(2):          # head pair
                    cps = [[ctxps.tile([65, 512], F32, tag="cacc",
                                       name=f"cacc{i}_{n}") for n in range(2)]
                           for i in range(2)]
                    for ks in range(ST):
                        ci = ks // MC
                        if hp == 0 and ks % MC == 0:
                            mch = maskp.tile([P, MC, NQ], F16, tag="mch",
                                             name=f"mch{qc}_{ci}")
                            nc.sync.dma_start(
                                mch[:],
                                maskT[ks * P:(ks + MC) * P,
                                      qc * NQ:(qc + 1) * NQ].rearrange(
                                    "(kt p) q -> p kt q", p=P),
                            )
                            mtiles[ci] = mch
                        mcur = mtiles[ci]
                        for hh in range(2):  # head within pair -> PE row group
                            h = 2 * hp + hh
                            st_ = stps.tile([P, NQ], F32, tag="st")
                            for n2 in range(2):
                                nc.tensor.matmul(
                                    st_[:, n2 * 512:(n2 + 1) * 512],
                                    khT[hh * 64:(hh + 1) * 64, hp,
                                        ks * P:(ks + 1) * P],
                                    qhT[hh * 64:(hh + 1) * 64, hp,
                                        qc * NQ + n2 * 512:qc * NQ + (n2 + 1) * 512],
                                    start=True, stop=True,
                                )
                            p_t = pwork.tile([P, NQ], F16, tag="pt")
                            nc.scalar.activation(p_t[:], st_[:], Exp, scale=0.125)
                            nc.vector.tensor_mul(p_t[:], p_t[:],
                                                 mcur[:, ks % MC, :])
                            for n2 in range(2):
                                nc.tensor.matmul(
                                    cps[hh][n2][:],
                                    vh[:, ks, h, :],
                                    p_t[:, n2 * 512:(n2 + 1) * 512],
                                    start=(ks == 0), stop=(ks == ST - 1),
                                )
                    for hh in range(2):
                        for n2 in range(2):
                            rr = small.tile([1, 512], F32, tag="rr", bufs=1,
                                            name=f"rr{hh}_{n2}")
                            nc.vector.reciprocal(rr[:], cps[hh][n2][64:65, :])
                            rb = small.tile([64, 512], F32, tag="rb",
                                            name=f"rb{hh}_{n2}")
                            nc.gpsimd.partition_broadcast(rb[:], rr[:])
                            nc.vector.tensor_mul(
                                ctxT[hh * 64:(hh + 1) * 64, hp,
                                     qc * NQ + n2 * 512:qc * NQ + (n2 + 1) * 512],
                                cps[hh][n2][0:64, :],
                                rb[:],
                            )
            ctxps_cm.__exit__(None, None, None)
            stps_cm.__exit__(None, None, None)

            # ---- phase C: output projection (partial) ----
            nc.sync.dma_start(wo_sb[:], woT.rearrange("(kt p) e -> p kt e", p=P))
            outps_cm = tc.tile_pool(name="outps", bufs=4, space="PSUM")
            outps = outps_cm.__enter__()
            for st in range(ST):
                ops = [outps.tile([P, 512], F32, tag="ops", name=f"ops{st}_{e}")
                       for e in range(2)]
                for ec in range(2):
                    for kt2 in range(2):
                        nc.tensor.matmul(
                            ops[ec][:],
                            ctxT[:, kt2, st * P:(st + 1) * P],
                            wo_sb[:, kt2, ec * 512:(ec + 1) * 512],
                            start=(kt2 == 0), stop=(kt2 == 1),
                        )
                o_sb = osb.tile([P, E], F16, tag="o")
                nc.scalar.activation(o_sb[:, 0:512], ops[0][:], Copy)
                nc.vector.tensor_copy(o_sb[:, 512:1024], ops[1][:])
                nc.sync.dma_start(out[st * P:(st + 1) * P, :], o_sb[:])
            outps_cm.__exit__(None, None, None)

    nc.compile()
    return nc


def kernel(q, k, v, mask, Wq, bq, Wk, bk, Wv, bv, Wo, bo):
    global LAST_RESULTS
    q = np.asarray(q, np.float32)
    k = np.asarray(k, np.float32)
    v = np.asarray(v, np.float32)
    mask = np.asarray(mask)
    Wq = np.asarray(Wq, np.float32)
    Wk = np.asarray(Wk, np.float32)
    Wv = np.asarray(Wv, np.float32)
    Wo = np.asarray(Wo, np.float32)
    bq = np.asarray(bq, np.float32)
    bk = np.asarray(bk, np.float32)
    bv = np.asarray(bv, np.float32)
    bo = np.asarray(bo, np.float32)

    use_bias_qk = bool(np.any(bq) or np.any(bk))
    use_bias_v = bool(np.any(bv))

    global LAST_NC
    nc = _build_program(use_bias_qk, use_bias_v)
    LAST_NC = nc

    xT = {}
    for b in range(B):
        xT[("q", b)] = _f16(q[b].T)
        xT[("k", b)] = _f16(k[b].T)
        xT[("v", b)] = _f16(v[b].T)
        xT[("m", b)] = _f16(mask[b, 0].T.astype(np.float32))

    in_maps = []
    for c in range(8):
        b, g = divmod(c, G)
        js = slice(g * J, (g + 1) * J)
        in_maps.append({
            "xqT": xT[("q", b)],
            "xkT": xT[("k", b)],
            "xvT": xT[("v", b)],
            "maskT": xT[("m", b)],
            "wqT": _f16(Wq[js, :].T),
            "wkT": _f16(Wk[js, :].T),
            "wvT": _f16(Wv[js, :].T),
            "woT": _round_f32r(Wo[:, js].T),
            "bq": np.ascontiguousarray(bq[js]),
            "bk": np.ascontiguousarray(bk[js]),
            "bv": np.ascontiguousarray(bv[js]),
        })

    os.environ["BASS_NEVER_TRACE"] = "1"
    res = bass_utils.run_bass_kernel_spmd(
        nc, in_maps, core_ids=list(range(8)), trace=False,
    )
    LAST_RESULTS = res

    full = np.zeros((B, S, E), np.float32)
    for c in range(8):
        b = c // G
        full[b] += res.results[c]["out"].astype(np.float32)
    full += bo[None, None, :]
    return full



# revision 35
# speedup vs baseline: 1.0376x; 1.0376x over previous
"""Multi-head attention on 8 NeuronCores (Trainium2, Bass/Tile).

Problem: B=2, S=2048, E=1024, H=16, D=64 MHA with int mask, fp32.

Sharding: core c = 4*b + g handles batch b, head group g (4 heads = a
256-wide slice of E).  Wo is row-sharded; the host sums the 4 partials
per batch and adds bo.

Device pipeline per core (all matmuls fp16 moving-side, fp32 PSUM):
  proj     : kt-outer accumulation (starts as soon as each 128-row DMA
             chunk of xT lands) -> qhT, khT [j, S] fp16 pair-major,
             vh [s, j] fp16 with a ones column (softmax denominator).
  scores   : [k, q] per (qc, ks) into PSUM [128, 4, 256] (2 banks).
  exp      : ACT exact exp for most ks-tiles; DVE Schraudolph
             (bits16 = s*184.665 + 15315 -> int16, bitcast fp16) for
             SCH_KS tiles per qc to offload the Activation engine.
  mask     : tensor-tensor multiply (fp16 2x mode on DVE; a share of
             ks-tiles goes to GPSIMD), mask tile broadcast across the
             4 heads via a stride-0 AP dim.
  ctx      : natural layout [q, 65] accumulated over ks in PSUM
             [128, 2, 4, 128-padded]; raw-evac fp16, then deferred
             per-partition reciprocal (DVE) + scale (GPSIMD).
  ctxT     : PE transposes [128, 64] -> [64, 128] via identity matmul,
             evac to [j, S] fp16; partial out = ctxT.T @ Wo on PE.
"""

import os
import sys

sys.path.insert(0, "/opt/trn_rl_repo")

import numpy as np

import concourse.mybir as mybir
import concourse.tile as tile
from concourse import bacc
from concourse import bass_utils

B, S, E, H = 2, 2048, 1024, 16
D = E // H              # 64
G = 4                   # head groups (cores per batch)
HL = H // G             # 4 local heads per core
J = HL * D              # 256 local j width
P = 128
KT = E // P             # 8 contract tiles for projections
ST = S // P             # 16 s-tiles / ks-tiles
NQ = 256                # q-chunk width for attention
QC = S // NQ            # 8 q chunks
QS = NQ // P            # 2 q-slices of 128 per chunk
MC = 4                  # mask ks-tiles per DMA chunk

# ks-tiles (per q-chunk) whose exp runs on DVE via Schraudolph bit trick
# (spread out so ACT and DVE interleave instead of serializing per qc).
SCH_KS = frozenset(
    int(x) for x in os.environ.get("KV_SCH", "2,6,10,14").split(",") if x != "")
# sch-ks mask-multiplies go to GPSIMD to keep DVE off the critical path.
POOL_MASK_KS = SCH_KS if os.environ.get("KV_POOLMASK", "1") == "1" else frozenset()
NORM_POOL = os.environ.get("KV_NORM_POOL", "1") == "1"
KS_ROT = os.environ.get("KV_KSROT", "1") == "1"
PHASE = int(os.environ.get("KV_PHASE", "3"))  # 1=proj 2=+attn 3=full
NOMASK = os.environ.get("KV_NOMASK", "0") == "1"
NONORM = os.environ.get("KV_NONORM", "0") == "1"
NOCTX = os.environ.get("KV_NOCTX", "0") == "1"
NOEVAC = os.environ.get("KV_NOEVAC", "0") == "1"
NOEXP = os.environ.get("KV_NOEXP", "0") == "1"
NOMDMA = os.environ.get("KV_NOMDMA", "0") == "1"
QCN = int(os.environ.get("KV_QC", "0")) or None
KSN = int(os.environ.get("KV_KSN", "0")) or None
SLOT = {0: 0, 2: 1, 1: 2, 3: 3}  # head -> score-psum slot

# Schraudolph fp16-bits constants: bits16 = round(s*A + B), p = bits16 as fp16
SCH_A = 1477.3195 * 0.125
SCH_B = 15360.0 - 45.0

F32 = mybir.dt.float32
F16 = mybir.dt.float16
I16 = mybir.dt.int16

# Exposed for test.py / bench.py.
LAST_RESULTS = None
LAST_NC = None


def _f16(x: np.ndarray) -> np.ndarray:
    return np.ascontiguousarray(x, dtype=np.float32).astype(np.float16)


def _build_program(use_bias_qk: bool, use_bias_v: bool):
    nc = bacc.Bacc("TRN2", target_bir_lowering=False, debug=False, num_devices=8)

    xqT = nc.dram_tensor("xqT", [E, S], F16, kind="ExternalInput")
    xkT = nc.dram_tensor("xkT", [E, S], F16, kind="ExternalInput")
    xvT = nc.dram_tensor("xvT", [E, S], F16, kind="ExternalInput")
    maskT = nc.dram_tensor("maskT", [S, S], F16, kind="ExternalInput")
    wqT = nc.dram_tensor("wqT", [E, J], F16, kind="ExternalInput")
    wkT = nc.dram_tensor("wkT", [E, J], F16, kind="ExternalInput")
    wvT = nc.dram_tensor("wvT", [E, J], F16, kind="ExternalInput")
    woT = nc.dram_tensor("woT", [J, E], F16, kind="ExternalInput")
    ident = nc.dram_tensor("ident", [P, P], F16, kind="ExternalInput")
    bq = nc.dram_tensor("bq", [J], F32, kind="ExternalInput")
    bk = nc.dram_tensor("bk", [J], F32, kind="ExternalInput")
    bv = nc.dram_tensor("bv", [J], F32, kind="ExternalInput")
    out = nc.dram_tensor("out", [S, E], F16, kind="ExternalOutput")

    Copy = mybir.ActivationFunctionType.Copy
    Exp = mybir.ActivationFunctionType.Exp
    MULT = mybir.AluOpType.mult
    ADD = mybir.AluOpType.add

    with tile.TileContext(nc) as tc:
        with (
            tc.tile_pool(name="consts", bufs=1) as consts,
            tc.tile_pool(name="persist", bufs=1) as persist,
            tc.tile_pool(name="xs", bufs=3) as xs,
            tc.tile_pool(name="maskp", bufs=2) as maskp,
            tc.tile_pool(name="pwork", bufs=6) as pwork,
            tc.tile_pool(name="osb", bufs=4) as osb,
            tc.tile_pool(name="small", bufs=2) as small,
        ):
            # ---- weights / constants ----
            wk_sb = consts.tile([P, KT, J], F16, tag="wk")
            wq_sb = consts.tile([P, KT, J], F16, tag="wq")
            wv_sb = consts.tile([P, KT, J], F16, tag="wv")
            wo_sb = consts.tile([P, J // P, E], F16, tag="wo")
            id_sb = consts.tile([P, P], F16, tag="ident")

            if use_bias_qk:
                bq_sb = consts.tile([P, J // P], F32, tag="bq")
                bk_sb = consts.tile([P, J // P], F32, tag="bk")
                nc.sync.dma_start(bq_sb[:], bq.rearrange("(pr p) -> p pr", p=P))
                nc.sync.dma_start(bk_sb[:], bk.rearrange("(pr p) -> p pr", p=P))
            if use_bias_v:
                bv_row = consts.tile([1, J], F32, tag="bvr")
                nc.sync.dma_start(bv_row[:], bv.rearrange("j -> 1 j"))
                bv_bc = consts.tile([P, J], F32, tag="bvb")
                nc.gpsimd.partition_broadcast(bv_bc[:], bv_row[:])

            # ---- persistent activations ----
            khT = persist.tile([P, 2, S], F16, tag="khT")
            qhT = persist.tile([P, 2, S], F16, tag="qhT")
            vh = persist.tile([P, ST, HL, 65], F16, tag="vh")
            ctx_raw = persist.tile([P, ST, HL, 65], F16, tag="craw")
            ctx_nrm = persist.tile([P, ST, HL, D], F16, tag="cnrm")
            ctxT = persist.tile([P, 2, S], F16, tag="ctxT")

            nc.gpsimd.memset(vh[:, :, :, 64:65], 1.0)

            # ---- phase A: projections, kt-outer (DMA-paced) ----
            projacc_cm = tc.tile_pool(name="projacc", bufs=8, space="PSUM")
            projacc = projacc_cm.__enter__()

            nc.sync.dma_start(wk_sb[:], wkT.rearrange("(kt p) j -> p kt j", p=P))
            nc.sync.dma_start(wq_sb[:], wqT.rearrange("(kt p) j -> p kt j", p=P))

            # khT then qhT: [j, s] pair-major; 8 accumulators (sc, pair)
            for w_sb, x_dram, outT, b_tag in (
                (wk_sb, xkT, khT, "bk"),
                (wq_sb, xqT, qhT, "bq"),
            ):
                accs = [projacc.tile([P, 512], F32, tag="pacc",
                                     name=f"pa{b_tag}{i}") for i in range(8)]
                for kt in range(KT):
                    xt = xs.tile([P, S], F16, tag="xt")
                    nc.sync.dma_start(xt[:], x_dram[kt * P:(kt + 1) * P, :])
                    for sc in range(4):
                        for pair in range(2):
                            nc.tensor.matmul(
                                accs[sc * 2 + pair][:],
                                w_sb[:, kt, pair * P:(pair + 1) * P],
                                xt[:, sc * 512:(sc + 1) * 512],
                                start=(kt == 0), stop=(kt == KT - 1),
                            )
                for sc in range(4):
                    for pair in range(2):
                        dst = outT[:, pair, sc * 512:(sc + 1) * 512]
                        src = accs[sc * 2 + pair][:]
                        if use_bias_qk:
                            bias = (bq_sb if b_tag == "bq" else bk_sb)[:, pair:pair + 1]
                            nc.scalar.activation(dst, src, Copy, bias=bias)
                        elif pair == 0:
                            nc.vector.tensor_copy(dst, src)
                        else:
                            nc.scalar.activation(dst, src, Copy)

            # vh: natural [s, j]; 8 banks hold 16 accumulators (2 per bank)
            vaccs = [projacc.tile([P, 2, J], F32, tag="pacc", name=f"va{t}")
                     for t in range(8)]
            nc.sync.dma_start(wv_sb[:], wvT.rearrange("(kt p) j -> p kt j", p=P))
            for kt in range(KT):
                xt = xs.tile([P, S], F16, tag="xt", name=f"xv{kt}")
                nc.sync.dma_start(xt[:], xvT[kt * P:(kt + 1) * P, :])
                for t in range(8):
                    for u in range(2):
                        st = 2 * t + u
                        nc.tensor.matmul(
                            vaccs[t][:, u, :],
                            xt[:, st * P:(st + 1) * P],
                            wv_sb[:, kt, :],
                            start=(kt == 0 and u == 0),
                            stop=(kt == KT - 1 and u == 1),
                            skip_group_check=True,
                        )
            for t in range(8):
                src3 = vaccs[t][:].rearrange("p u (h d) -> p u h d", h=HL)
                dst = vh[:, 2 * t:2 * t + 2, :, 0:64]
                if use_bias_v:
                    bvb = (bv_bc[:].rearrange("p (h d) -> p h d", h=HL)
                           .rearrange("p (u2 h) d -> p u2 h d", u2=1)
                           .broadcast_to([P, 2, HL, 64]))
                    nc.vector.tensor_tensor(dst, src3, bvb, ADD)
                elif t % 2 == 0:
                    nc.vector.tensor_copy(dst, src3)
                else:
                    nc.scalar.activation(dst, src3, Copy)

            nc.sync.dma_start(id_sb[:], ident[:, :])
            projacc_cm.__exit__(None, None, None)

            # ---- phase B: attention ----
            if PHASE < 2:
                nc.vector.tensor_copy(ctx_nrm[:, 0, :, :], vh[:, 0, :, 0:64])
            stps_cm = tc.tile_pool(name="stps", bufs=3, space="PSUM")
            stps = stps_cm.__enter__()
            ctxps_cm = tc.tile_pool(name="ctxps", bufs=2, space="PSUM")
            ctxps = ctxps_cm.__enter__()

            KS_ORDER = (list(range(4, ST)) + list(range(4))) if KS_ROT else list(range(ST))
            pend_ctx = []  # deferred ctx matmuls (2-deep software pipeline)

            def flush_ctx(depth):
                while len(pend_ctx) > depth:
                    p_t, cps, ks = pend_ctx.pop(0)
                    if NOCTX:
                        if ks == KS_ORDER[0]:
                            for qs in range(QS):
                                nc.tensor.matmul(
                                    cps[qs][:, 0, 0:65], p_t[:, 0, 0:P],
                                    vh[:, ks, 0, :], start=True, stop=True,
                                    skip_group_check=True)
                        continue
                    for qs in range(QS):
                        for h in range(HL):
                            # one accumulation group per 2KB PSUM region: only
                            # the region's first write starts (marks the whole
                            # region pending-zero); later heads' first writes
                            # auto-zero on first touch.
                            nc.tensor.matmul(
                                cps[qs][:, h, 0:65],
                                p_t[:, SLOT[h], qs * P:(qs + 1) * P],
                                vh[:, ks, h, :],
                                start=(ks == KS_ORDER[0] and h == 0),
                                stop=(ks == KS_ORDER[-1] and h == HL - 1),
                                skip_group_check=True,
                            )

            norm_work = []  # deferred (stt) normalize units
            NRM_SPLIT = 2   # every qc boundary, drain this many stt units

            def drain_norm(n):
                done = 0
                while norm_work and done < n:
                    stt = norm_work.pop(0)
                    if NONORM:
                        done += 1
                        continue
                    rr = small.tile([P, HL, 1], F32, tag="rr", name=f"rr{stt}")
                    nc.vector.reciprocal(rr[:], ctx_raw[:, stt, :, 64:65])
                    for h in range(HL):
                        eng_n = nc.gpsimd if NORM_POOL else nc.vector
                        eng_n.tensor_scalar(
                            ctx_nrm[:, stt, h, :],
                            ctx_raw[:, stt, h, 0:64],
                            rr[:, h, :], None, MULT,
                        )
                    done += 1

            for qc in range(min(QC, QCN or QC) if PHASE >= 2 else 0):
                cps = [ctxps.tile([P, HL, P], F32, tag="cacc",
                                  name=f"cacc{qc}_{qs}") for qs in range(QS)]
                mtiles = {}
                for ks in (KS_ORDER[:KSN] if KSN else KS_ORDER):
                    ci = ks // MC
                    if ks % MC == 0 and not NOMDMA:
                        mch = maskp.tile([P, MC, NQ], F16, tag="mch",
                                         name=f"mch{qc}_{ci}")
                        nc.sync.dma_start(
                            mch[:],
                            maskT[ks * P:(ks + MC) * P,
                                  qc * NQ:(qc + 1) * NQ].rearrange(
                                "(kt p) q -> p kt q", p=P),
                        )
                        mtiles[ci] = mch
                    mcur = mtiles.get(ci)

                    st_ = stps.tile([P, HL, NQ], F32, tag="st")
                    # slot order groups same PE tile_position per PSUM bank:
                    # bank A = heads 0,2 (partitions 0-63), bank B = heads 1,3
                    # (partitions 64-127). Mixing positions within one bank
                    # faults the device.
                    for h in (0, 2, 1, 3):
                        hp, hh = h // 2, h % 2
                        nc.tensor.matmul(
                            st_[:, SLOT[h], :],
                            khT[hh * 64:(hh + 1) * 64, hp, ks * P:(ks + 1) * P],
                            qhT[hh * 64:(hh + 1) * 64, hp,
                                qc * NQ:(qc + 1) * NQ],
                            start=True, stop=True,
                        )
                    p_t = pwork.tile([P, HL, NQ], F16, tag="pt")
                    if NOEXP:
                        nc.vector.tensor_copy(p_t[:], st_[:])
                    elif ks in SCH_KS:
                        nc.vector.tensor_scalar(
                            p_t[:].bitcast(I16), st_[:], SCH_A, SCH_B, MULT, ADD
                        )
                    elif True:
                        nc.scalar.activation(p_t[:], st_[:], Exp, scale=0.125)
                    # mask multiply, mask tile broadcast across heads (stride 0)
                    if not NOMASK:
                        mb = (mcur[:, ks % MC, :]
                              .rearrange("p (u q) -> p u q", u=1)
                              .broadcast_to([P, HL, NQ]))
                        eng = nc.gpsimd if ks in POOL_MASK_KS else nc.vector
                        eng.tensor_tensor(p_t[:], p_t[:], mb, MULT)

                    flush_ctx(2)
                    pend_ctx.append((p_t, cps, ks))
                flush_ctx(0)

                # raw evacuation (frees ctx psum for next qc)
                for qs in range(QS):
                    if not NOEVAC:
                        nc.vector.tensor_copy(
                            ctx_raw[:, qc * QS + qs, :, :],
                            cps[qs][:, :, 0:65],
                        )
                    norm_work.append(qc * QS + qs)
                drain_norm(NRM_SPLIT)
            drain_norm(len(norm_work))

            ctxps_cm.__exit__(None, None, None)
            stps_cm.__exit__(None, None, None)

            # ---- phase C: transpose ctx -> [j, s] and output projection ----
            trps_cm = tc.tile_pool(name="trps", bufs=3, space="PSUM")
            trps = trps_cm.__enter__()
            outps_cm = tc.tile_pool(name="outps", bufs=2, space="PSUM")
            outps = outps_cm.__enter__()

            nc.sync.dma_start(wo_sb[:], woT.rearrange("(hp p) e -> p hp e", p=P))

            def emit_transpose(stt):
                tr = trps.tile([64, HL, P], F16, tag="tr", name=f"tr{stt}")
                for h in range(HL):
                    nc.tensor.matmul(
                        tr[:, h, :],
                        ctx_nrm[:, stt, h, :],
                        id_sb[:],
                        is_transpose=True,
                    )
                sl = slice(stt * P, (stt + 1) * P)
                for lo in range(2):  # partition half: heads (lo, lo+2)
                    nc.vector.tensor_copy(ctxT[64 * lo:64 * lo + 64, :, sl],
                                          tr[:, lo::2, :])

            def emit_outproj(stt):
                sl = slice(stt * P, (stt + 1) * P)
                ops = outps.tile([P, E], F32, tag="ops", name=f"ops{stt}")
                for ec in range(2):
                    for hp in range(2):
                        nc.tensor.matmul(
                            ops[:, ec * 512:(ec + 1) * 512],
                            ctxT[:, hp, sl],
                            wo_sb[:, hp, ec * 512:(ec + 1) * 512],
                            start=(hp == 0), stop=(hp == 1),
                        )
                o_sb = osb.tile([P, E], F16, tag="o")
                nc.scalar.activation(o_sb[:], ops[:], Copy)
                nc.sync.dma_start(out[stt * P:(stt + 1) * P, :], o_sb[:])

            if PHASE >= 3:
                emit_transpose(0)
                emit_transpose(1)
                for stt in range(ST):
                    if stt + 2 < ST:
                        emit_transpose(stt + 2)
                    emit_outproj(stt)
            else:
                o_sb0 = osb.tile([P, E], F16, tag="o")
                nc.scalar.activation(
                    o_sb0[:, 0:E // 2],
                    khT[:, 0, 0:E // 2], mybir.ActivationFunctionType.Copy)
                for stt in range(ST):
                    nc.sync.dma_start(out[stt * P:(stt + 1) * P, :], o_sb0[:])
            outps_cm.__exit__(None, None, None)
            trps_cm.__exit__(None, None, None)

    nc.compile()
    return nc


def kernel(q, k, v, mask, Wq, bq, Wk, bk, Wv, bv, Wo, bo):
    global LAST_RESULTS, LAST_NC
    q = np.asarray(q, np.float32)
    k = np.asarray(k, np.float32)
    v = np.asarray(v, np.float32)
    mask = np.asarray(mask)
    Wq = np.asarray(Wq, np.float32)
    Wk = np.asarray(Wk, np.float32)
    Wv = np.asarray(Wv, np.float32)
    Wo = np.asarray(Wo, np.float32)
    bq = np.asarray(bq, np.float32)
    bk = np.asarray(bk, np.float32)
    bv = np.asarray(bv, np.float32)
    bo = np.asarray(bo, np.float32)

    use_bias_qk = bool(np.any(bq) or np.any(bk))
    use_bias_v = bool(np.any(bv))

    nc = _build_program(use_bias_qk, use_bias_v)
    LAST_NC = nc

    identity = np.eye(P, dtype=np.float16)
    xT = {}
    for b in range(B):
        xT[("q", b)] = _f16(q[b].T)
        xT[("k", b)] = _f16(k[b].T)
        xT[("v", b)] = _f16(v[b].T)
        xT[("m", b)] = _f16(mask[b, 0].T.astype(np.float32))

    in_maps = []
    for c in range(8):
        b, g = divmod(c, G)
        js = slice(g * J, (g + 1) * J)
        in_maps.append({
            "xqT": xT[("q", b)],
            "xkT": xT[("k", b)],
            "xvT": xT[("v", b)],
            "maskT": xT[("m", b)],
            "wqT": _f16(Wq[js, :].T),
            "wkT": _f16(Wk[js, :].T),
            "wvT": _f16(Wv[js, :].T),
            "woT": _f16(Wo[:, js].T),
            "ident": identity,
            "bq": np.ascontiguousarray(bq[js]),
            "bk": np.ascontiguousarray(bk[js]),
            "bv": np.ascontiguousarray(bv[js]),
        })

    os.environ["BASS_NEVER_TRACE"] = "1"
    res = bass_utils.run_bass_kernel_spmd(
        nc, in_maps, core_ids=list(range(8)), trace=False,
    )
    LAST_RESULTS = res

    full = np.zeros((B, S, E), np.float32)
    for c in range(8):
        b = c // G
        full[b] += res.results[c]["out"].astype(np.float32)
    full += bo[None, None, :]
    return full
